# revision 4
# baseline (speedup 1.0000x reference)
"""Trainium2 Bass kernel for nn_BezierGlyph (retrieval_knn).

Math (matching the jax reference):
  pts  = cubic-bezier samples of clip(control_points, 0, 1)   # [512, 2]
  d_ij = |pixel_i - pts_j|
  m_i  = -logsumexp(-256 * d_i:) / 256                        # softmin
  out  = 1 - sigmoid((0.04 - m) * 200)                        # (1, 512, 512)

Strategy (sharding_hint: shard pixels, replicate points):
  * 512x512 pixels split into 256 blocks of 32x32; each block only needs
    sample points within 0.2 of its bbox (farther points contribute less
    than 5e-9 relative to the softmin sum wherever the output is not
    exactly 1.0f; dropping them only biases far-pixel sums DOWN, which
    keeps those outputs at exactly 1.0f).
  * Blocks are LPT-balanced across the 8 cores (32 blocks each). The SPMD
    program is shared, so per-slot candidate capacity K_sched[i] is the
    max across cores of each core's i-th largest padded candidate count.
  * Per block the device computes, via PE matmul (contraction K=4):
        S = |p|^2 - 2 p.q + |q|^2 = dist^2
    then on the Scalar engine with a single activation-table set
    (natural_log_exp_and_others; no table thrashing):
        u = ln(S + 1e-6)            # 1e-6 guards fp32-negative dist^2
        v = exp(0.5*u + ln(256))    # = 256 * d
        w = exp(-v)                 # = exp(-256 d)
    row-sums on the Vector engine, then per block-group:
        t = 8 + 0.78125 * ln(sum + 1e-37)
        out = 1 / (1 + exp(t))      # = 1 - sigmoid(-t)
"""

import math

import numpy as np

import concourse.bass as bass
import concourse.tile as tile
from concourse import bacc, mybir
from concourse.bass_utils import run_bass_kernel_spmd
from concourse.masks import make_identity

SIZE = 512
N_SAMPLES = 32
N_STROKES = 16
NPTS = N_STROKES * N_SAMPLES  # 512
SHARP = float(N_SAMPLES) * 8.0  # 256
STROKE_WIDTH = 0.04
OUT_SCALE = 8.0 / STROKE_WIDTH  # 200

NCORES = 8
BLK = 32  # block side in pixels
NB = SIZE // BLK  # 16 blocks per image side
NBLOCKS = NB * NB  # 256
BLOCKS_PER_CORE = NBLOCKS // NCORES  # 32
PXB = BLK * BLK  # 1024 pixels per block
SUBT = PXB // 128  # 8 subtiles of 128 pixels
CUTOFF = 0.2  # candidate radius from block bbox
PADG = 32  # candidate count granularity
DUMMY = (3.0, 3.0)  # far-away pad point: exp(-256*d) == 0 in fp32

f32 = mybir.dt.float32
AF = mybir.ActivationFunctionType

_prog_cache: dict = {}


def _bezier_points(control_points: np.ndarray) -> np.ndarray:
    """[16,4,2] control points -> [512,2] float64 curve samples."""
    pts = np.clip(control_points.astype(np.float64), 0.0, 1.0)
    t = np.linspace(0.0, 1.0, N_SAMPLES)[None, :, None]
    mt = 1.0 - t
    p0, p1, p2, p3 = (pts[:, k : k + 1, :] for k in range(4))
    cur = mt**3 * p0 + 3 * mt**2 * t * p1 + 3 * mt * t**2 * p2 + t**3 * p3
    return cur.reshape(-1, 2)


def _build_program(k_sched: tuple[int, ...]):
    """Build + compile the SPMD Bass program for a fixed per-slot candidate
    schedule. Returns (nc, mov_offsets)."""
    nslots = len(k_sched)
    ngroups = nslots // 16
    mov_off = np.concatenate([[0], np.cumsum(k_sched)]).astype(int)
    mov_total = int(mov_off[-1])

    nc = bacc.Bacc(None, target_bir_lowering=False)

    pix_d = nc.dram_tensor("pix", [4, nslots * PXB], f32, kind="ExternalInput")
    mov_d = nc.dram_tensor("mov", [4, mov_total], f32, kind="ExternalInput")
    out_d = nc.dram_tensor("out", [nslots * SUBT, 128], f32, kind="ExternalOutput")

    ln256 = math.log(SHARP)

    with tile.TileContext(nc) as tc:
        with (
            tc.tile_pool(name="io", bufs=1) as io,
            tc.tile_pool(name="work", bufs=3) as work,
            tc.tile_pool(name="acc", bufs=2) as acc,
            tc.tile_pool(name="fin", bufs=2) as fin,
            tc.tile_pool(name="psum", bufs=2, space="PSUM") as psum,
        ):
            ident = io.tile([128, 128], f32)
            make_identity(nc, ident)
            b_eps = io.tile([128, 1], f32)
            nc.vector.memset(b_eps, 1e-6)
            b_ln256 = io.tile([128, 1], f32)
            nc.vector.memset(b_ln256, ln256)
            b_tiny = io.tile([128, 1], f32)
            nc.vector.memset(b_tiny, 1e-37)
            b_eight = io.tile([128, 1], f32)
            nc.vector.memset(b_eight, STROKE_WIDTH * OUT_SCALE)
            mov_all = io.tile([4, mov_total], f32)
            nc.gpsimd.dma_start(mov_all[:], mov_d[:])
            pix_all = io.tile([4, nslots * PXB], f32)
            # chunked load so the first blocks start sooner
            pchunk = (nslots // 4) * PXB
            for ci in range(4):
                nc.gpsimd.dma_start(
                    pix_all[:, ci * pchunk : (ci + 1) * pchunk],
                    pix_d[:, ci * pchunk : (ci + 1) * pchunk],
                )

            for g in range(ngroups):
                sums = acc.tile([128, 128], f32, tag="sums")
                for bi in range(16):
                    i = g * 16 + bi
                    K = k_sched[i]
                    mov = mov_all[:, mov_off[i] : mov_off[i] + K]
                    ut = work.tile([128, SUBT, 512], f32, tag="u")
                    for w in range(2):
                        pt = psum.tile([128, 4, 512], f32, tag="ps")
                        for s in range(4):
                            st = w * 4 + s
                            nc.tensor.matmul(
                                pt[:, s, :K],
                                pix_all[:, i * PXB + st * 128 : i * PXB + (st + 1) * 128],
                                mov,
                                start=True,
                                stop=True,
                            )
                        # u = ln(dist^2 + 1e-6)
                        nc.scalar.activation(
                            ut[:, w * 4 : (w + 1) * 4, :K],
                            pt[:, :, :K],
                            AF.Ln,
                            bias=b_eps[:],
                        )
                    # v = exp(0.5 u + ln 256) = 256 d
                    nc.scalar.activation(
                        ut[:, :, :K], ut[:, :, :K], AF.Exp, bias=b_ln256[:], scale=0.5
                    )
                    # w = exp(-v)
                    nc.scalar.activation(ut[:, :, :K], ut[:, :, :K], AF.Exp, scale=-1.0)
                    # per-pixel sums over the K candidates
                    nc.vector.reduce_sum(
                        sums[:, bi * SUBT : (bi + 1) * SUBT],
                        ut[:, :, :K],
                        axis=mybir.AxisListType.X,
                    )
                # t = 8 + 0.78125 * ln(sum + 1e-37); out = 1/(1 + exp(t))
                zt = fin.tile([128, 128], f32, tag="z")
                nc.scalar.activation(zt[:], sums[:], AF.Ln, bias=b_tiny[:])
                nc.scalar.activation(
                    zt[:], zt[:], AF.Exp, bias=b_eight[:], scale=OUT_SCALE / SHARP,
                )
                nc.vector.tensor_scalar_add(zt[:], zt[:], 1.0)
                nc.vector.reciprocal(zt[:], zt[:])
                # transpose so each output row is one subtile's 128 pixels
                ptt = psum.tile([128, 4, 512], f32, tag="ps")
                tview = ptt[:, 0, :128]
                nc.tensor.transpose(tview, zt[:], ident[:])
                ot = fin.tile([128, 128], f32, tag="o")
                nc.vector.tensor_copy(ot[:], tview)
                nc.sync.dma_start(out_d[g * 128 : (g + 1) * 128, :], ot[:])

    nc.compile()
    return nc, mov_off


def kernel(control_points: np.ndarray, pixel_grid: np.ndarray) -> np.ndarray:
    control_points = np.asarray(control_points, dtype=np.float32)
    pixel_grid = np.asarray(pixel_grid, dtype=np.float32)

    pts64 = _bezier_points(control_points)  # [512, 2] f64
    q32 = pts64.astype(np.float32)
    qn2 = (pts64[:, 0] ** 2 + pts64[:, 1] ** 2).astype(np.float32)

    # ---- block geometry from the actual pixel grid ----
    pg = pixel_grid.reshape(SIZE, SIZE, 2)
    # [NB, NB, BLK, BLK, 2] -> blocks (by, bx), local (lr, lc)
    pblk = pg.reshape(NB, BLK, NB, BLK, 2).transpose(0, 2, 1, 3, 4)
    pblk = np.ascontiguousarray(pblk).reshape(NBLOCKS, PXB, 2)
    bxmin = pblk[:, :, 0].min(1)
    bxmax = pblk[:, :, 0].max(1)
    bymin = pblk[:, :, 1].min(1)
    bymax = pblk[:, :, 1].max(1)

    # distance from each sample point to each block bbox
    dx = np.maximum(np.maximum(bxmin[:, None] - pts64[None, :, 0],
                               pts64[None, :, 0] - bxmax[:, None]), 0.0)
    dy = np.maximum(np.maximum(bymin[:, None] - pts64[None, :, 1],
                               pts64[None, :, 1] - bymax[:, None]), 0.0)
    cand = dx * dx + dy * dy < (CUTOFF + 1e-3) ** 2  # [NBLOCKS, 512]
    kcnt = cand.sum(1)
    kpad = np.maximum(((kcnt + PADG - 1) // PADG) * PADG, PADG).astype(int)

    # ---- LPT assignment: exactly BLOCKS_PER_CORE blocks per core ----
    order = np.argsort(-kpad, kind="stable")
    loads = np.zeros(NCORES)
    counts = np.zeros(NCORES, dtype=int)
    assign = np.zeros(NBLOCKS, dtype=int)
    for b in order:
        elig = np.flatnonzero(counts < BLOCKS_PER_CORE)
        c = elig[np.argmin(loads[elig])]
        assign[b] = c
        loads[c] += kpad[b]
        counts[c] += 1

    # per-core slots sorted by descending kpad; shared schedule = slotwise max
    core_blocks = []
    for c in range(NCORES):
        blks = np.flatnonzero(assign == c)
        blks = blks[np.argsort(-kpad[blks], kind="stable")]
        core_blocks.append(blks)
    core_blocks = np.stack(core_blocks)  # [8, 32]
    k_sched = tuple(int(kpad[core_blocks[:, i]].max()) for i in range(BLOCKS_PER_CORE))

    if k_sched not in _prog_cache:
        _prog_cache.clear()
        _prog_cache[k_sched] = _build_program(k_sched)
    nc, mov_off = _prog_cache[k_sched]
    mov_total = int(mov_off[-1])

    # ---- per-core input arrays ----
    in_maps = []
    for c in range(NCORES):
        pix = np.empty((4, BLOCKS_PER_CORE * PXB), dtype=np.float32)
        mov = np.empty((4, mov_total), dtype=np.float32)
        mov[0] = DUMMY[0]
        mov[1] = DUMMY[1]
        mov[2] = 1.0
        mov[3] = DUMMY[0] ** 2 + DUMMY[1] ** 2
        for i, b in enumerate(core_blocks[c]):
            px = pblk[b]  # [1024, 2] f32
            sl = slice(i * PXB, (i + 1) * PXB)
            pix[0, sl] = -2.0 * px[:, 0]
            pix[1, sl] = -2.0 * px[:, 1]
            pix[2, sl] = (px[:, 0].astype(np.float64) ** 2
                          + px[:, 1].astype(np.float64) ** 2).astype(np.float32)
            pix[3, sl] = 1.0
            idx = np.flatnonzero(cand[b])
            o = int(mov_off[i])
            mov[0, o : o + len(idx)] = q32[idx, 0]
            mov[1, o : o + len(idx)] = q32[idx, 1]
            mov[2, o : o + len(idx)] = 1.0
            mov[3, o : o + len(idx)] = qn2[idx]
        in_maps.append({"pix": pix, "mov": mov})

    global _last_in_maps
    _last_in_maps = in_maps
    res = run_bass_kernel_spmd(nc, in_maps, core_ids=list(range(NCORES)))

    # ---- unshard: scatter block results back into the image ----
    img = np.empty(SIZE * SIZE, dtype=np.float32)
    rows = np.arange(NB * BLK).reshape(NB, BLK)
    # flat indices of each block's 1024 pixels, in device pixel order
    by, bx = np.meshgrid(np.arange(NB), np.arange(NB), indexing="ij")
    lr, lc = np.meshgrid(np.arange(BLK), np.arange(BLK), indexing="ij")
    flat = ((by.reshape(-1, 1) * BLK + lr.reshape(-1)[None, :]) * SIZE
            + bx.reshape(-1, 1) * BLK + lc.reshape(-1)[None, :])  # [NBLOCKS, PXB]
    for c in range(NCORES):
        o = res.results[c]["out"].reshape(BLOCKS_PER_CORE, PXB)
        for i, b in enumerate(core_blocks[c]):
            img[flat[b]] = o[i]
    return img.reshape(1, SIZE, SIZE)


# revision 6
# speedup vs baseline: 2.2262x; 2.2262x over previous
"""Trainium2 Bass kernel for nn_BezierGlyph (retrieval_knn).

Math (matching the jax reference):
  pts  = cubic-bezier samples of clip(control_points, 0, 1)   # [512, 2]
  d_ij = |pixel_i - pts_j|
  m_i  = -logsumexp(-256 * d_i:) / 256                        # softmin
  out  = 1 - sigmoid((0.04 - m) * 200)                        # (1, 512, 512)

Strategy (sharding_hint: shard pixels, replicate points):
  * 512x512 pixels split into 256 blocks of 32x32; each block only needs
    sample points within 0.2 of its bbox (farther points contribute less
    than 1e-5 relative to the softmin sum wherever the output is not
    exactly 1.0f; dropping them only biases far-pixel sums DOWN, which
    keeps those outputs at exactly 1.0f).
  * Blocks are LPT-balanced across the 8 cores (32 blocks each). The SPMD
    program is shared, so per-slot candidate capacity K_sched[i] is the
    max across cores of each core's i-th largest padded candidate count.
  * dist^2 = |p|^2 - 2 p.q + |q|^2 via one PE matmul with an 18-row bf16
    contraction: each fp32 factor is split into 3 bf16 limbs (exact),
    bf16xbf16 products are exact in the fp32 PSUM accumulator, and limb
    products below 2^-24 are dropped. 4x faster than fp32 matmul at the
    same effective precision.
  * Scalar engine uses a single activation-table set
    (natural_log_exp_and_others; a post-compile pass dedups the
    per-function table reloads the stock pass inserts):
        u = ln(max(dist^2, 1e-8))   # max() on DVE kills fp32-negative noise
        v = exp(0.5*u + ln(256))    # = 256 * d
        w = exp(-v)                 # = exp(-256 d)
    row-sums on the Vector engine, then per 16-block group:
        t = 8 + 0.78125 * ln(sum + 1e-37)
        out = 1 / (1 + exp(t))      # = 1 - sigmoid(-t)
"""

import math

import ml_dtypes
import numpy as np

import concourse.bass as bass
import concourse.tile as tile
from concourse import bacc, mybir
from concourse.bass_utils import run_bass_kernel_spmd
from concourse.hw_specs import get_activation_tables
from concourse.masks import make_identity

SIZE = 512
N_SAMPLES = 32
N_STROKES = 16
NPTS = N_STROKES * N_SAMPLES  # 512
SHARP = float(N_SAMPLES) * 8.0  # 256
STROKE_WIDTH = 0.04
OUT_SCALE = 8.0 / STROKE_WIDTH  # 200

NCORES = 8
BLK = 32  # block side in pixels
NB = SIZE // BLK  # 16 blocks per image side
NBLOCKS = NB * NB  # 256
BLOCKS_PER_CORE = NBLOCKS // NCORES  # 32
PXB = BLK * BLK  # 1024 pixels per block
SUBT = PXB // 128  # 8 subtiles of 128 pixels
CUTOFF = 0.2  # candidate radius from block bbox
PADG = 32  # candidate count granularity
DUMMY = (3.0, 3.0)  # far-away pad point: exp(-256*d) == 0 in fp32
KROWS = 18  # bf16 limb-product rows in the matmul contraction

f32 = mybir.dt.float32
bf16 = mybir.dt.bfloat16
np_bf16 = ml_dtypes.bfloat16
AF = mybir.ActivationFunctionType

_prog_cache: dict = {}


def _bezier_points(control_points: np.ndarray) -> np.ndarray:
    """[16,4,2] control points -> [512,2] float64 curve samples."""
    pts = np.clip(control_points.astype(np.float64), 0.0, 1.0)
    t = np.linspace(0.0, 1.0, N_SAMPLES)[None, :, None]
    mt = 1.0 - t
    p0, p1, p2, p3 = (pts[:, k : k + 1, :] for k in range(4))
    cur = mt**3 * p0 + 3 * mt**2 * t * p1 + 3 * mt * t**2 * p2 + t**3 * p3
    return cur.reshape(-1, 2)


def _split3(x: np.ndarray):
    """fp32-exact 3-way bf16 limb split (f64 in, 3x bf16 out)."""
    a = x.astype(np_bf16)
    r = x - a.astype(np.float64)
    b = r.astype(np_bf16)
    r = r - b.astype(np.float64)
    c = r.astype(np_bf16)
    return a, b, c


def _limb_rows(v1, v2, v3, w1, w2, w3, scale=1.0):
    """The 6 (stationary, moving) limb pairs covering v*w to ~2^-24:
    v1w1, v1w2, v2w1, v2w2, v1w3, v3w1."""
    sv = [v1, v1, v2, v2, v1, v3]
    sw = [w1, w2, w1, w2, w3, w1]
    if scale != 1.0:
        sv = [(s.astype(np.float64) * scale).astype(np_bf16) for s in sv]
    return sv, sw


def _build_program(k_sched: tuple[int, ...]):
    """Build + compile the SPMD Bass program for a fixed per-slot candidate
    schedule. Returns (nc, mov_offsets)."""
    nslots = len(k_sched)
    ngroups = nslots // 16
    mov_off = np.concatenate([[0], np.cumsum(k_sched)]).astype(int)
    mov_total = int(mov_off[-1])

    nc = bacc.Bacc(None, target_bir_lowering=False)

    pix_d = nc.dram_tensor("pix", [KROWS, nslots * PXB], bf16, kind="ExternalInput")
    mov_d = nc.dram_tensor("mov", [KROWS, mov_total], bf16, kind="ExternalInput")
    out_d = nc.dram_tensor("out", [nslots * SUBT, 128], f32, kind="ExternalOutput")

    ln256 = math.log(SHARP)

    with tile.TileContext(nc) as tc:
        with (
            tc.tile_pool(name="io", bufs=1) as io,
            tc.tile_pool(name="work", bufs=3) as work,
            tc.tile_pool(name="acc", bufs=2) as acc,
            tc.tile_pool(name="fin", bufs=2) as fin,
            tc.tile_pool(name="psum", bufs=2, space="PSUM") as psum,
        ):
            ident = io.tile([128, 128], f32)
            make_identity(nc, ident)
            b_ln256 = io.tile([128, 1], f32)
            nc.vector.memset(b_ln256, ln256)
            b_tiny = io.tile([128, 1], f32)
            nc.vector.memset(b_tiny, 1e-37)
            b_eight = io.tile([128, 1], f32)
            nc.vector.memset(b_eight, STROKE_WIDTH * OUT_SCALE)
            mov_all = io.tile([KROWS, mov_total], bf16)
            nc.gpsimd.dma_start(mov_all[:], mov_d[:])
            pix_all = io.tile([KROWS, nslots * PXB], bf16)
            # chunked load so the first blocks start sooner
            pchunk = (nslots // 4) * PXB
            for ci in range(4):
                nc.gpsimd.dma_start(
                    pix_all[:, ci * pchunk : (ci + 1) * pchunk],
                    pix_d[:, ci * pchunk : (ci + 1) * pchunk],
                )

            for g in range(ngroups):
                sums = acc.tile([128, 128], f32, tag="sums")
                for bi in range(16):
                    i = g * 16 + bi
                    K = k_sched[i]
                    mov = mov_all[:, mov_off[i] : mov_off[i] + K]
                    ut = work.tile([128, SUBT, 512], f32, tag="u")
                    for w in range(2):
                        pt = psum.tile([128, 4, 512], f32, tag="ps")
                        for s in range(4):
                            st = w * 4 + s
                            nc.tensor.matmul(
                                pt[:, s, :K],
                                pix_all[:, i * PXB + st * 128 : i * PXB + (st + 1) * 128],
                                mov,
                                start=True,
                                stop=True,
                            )
                        # clamp away fp32 rounding noise (negative dist^2)
                        nc.vector.tensor_scalar_max(
                            ut[:, w * 4 : (w + 1) * 4, :K], pt[:, :, :K], 1e-8
                        )
                    # u = ln(dist^2)
                    nc.scalar.activation(ut[:, :, :K], ut[:, :, :K], AF.Ln)
                    # v = exp(0.5 u + ln 256) = 256 d
                    nc.scalar.activation(
                        ut[:, :, :K], ut[:, :, :K], AF.Exp, bias=b_ln256[:], scale=0.5
                    )
                    # w = exp(-v)
                    nc.scalar.activation(ut[:, :, :K], ut[:, :, :K], AF.Exp, scale=-1.0)
                    # per-pixel sums over the K candidates
                    nc.vector.reduce_sum(
                        sums[:, bi * SUBT : (bi + 1) * SUBT],
                        ut[:, :, :K],
                        axis=mybir.AxisListType.X,
                    )
                # t = 8 + 0.78125 * ln(sum + 1e-37); out = 1/(1 + exp(t))
                zt = fin.tile([128, 128], f32, tag="z")
                nc.scalar.activation(zt[:], sums[:], AF.Ln, bias=b_tiny[:])
                nc.scalar.activation(
                    zt[:], zt[:], AF.Exp, bias=b_eight[:], scale=OUT_SCALE / SHARP,
                )
                nc.vector.tensor_scalar_add(zt[:], zt[:], 1.0)
                nc.vector.reciprocal(zt[:], zt[:])
                # transpose so each output row is one subtile's 128 pixels
                ptt = psum.tile([128, 4, 512], f32, tag="ps")
                tview = ptt[:, 0, :128]
                nc.tensor.transpose(tview, zt[:], ident[:])
                ot = fin.tile([128, 128], f32, tag="o")
                nc.vector.tensor_copy(ot[:], tview)
                nc.sync.dma_start(out_d[g * 128 : (g + 1) * 128, :], ot[:])

    nc.compile()

    # Dedup activation-table loads: every Ln/Exp in this kernel is served by
    # the one combined set, so keep the first load (retargeted to it) and
    # drop the rest.
    combined_id = None
    for idx, (name, funcs) in enumerate(get_activation_tables(nc.m.arch).items()):
        if {AF.Ln, AF.Exp} <= funcs:
            combined_id = idx
            break
    assert combined_id is not None, "no activation table set with both Ln and Exp"
    for blk in nc.m.functions[0].blocks:
        loads = [i for i in blk.instructions
                 if isinstance(i, mybir.InstLoadActFuncSet)]
        if not loads:
            continue
        loads[0].act_func_set_id = combined_id
        for l in loads[1:]:
            blk.instructions.remove(l)

    return nc, mov_off


def kernel(control_points: np.ndarray, pixel_grid: np.ndarray) -> np.ndarray:
    control_points = np.asarray(control_points, dtype=np.float32)
    pixel_grid = np.asarray(pixel_grid, dtype=np.float32)

    pts64 = _bezier_points(control_points)  # [512, 2] f64
    q64 = pts64.astype(np.float32).astype(np.float64)  # the fp32 values, exactly
    qn64 = q64[:, 0] ** 2 + q64[:, 1] ** 2

    # ---- block geometry from the actual pixel grid ----
    pg = pixel_grid.reshape(SIZE, SIZE, 2)
    # [NB, NB, BLK, BLK, 2] -> blocks (by, bx), local (lr, lc)
    pblk = pg.reshape(NB, BLK, NB, BLK, 2).transpose(0, 2, 1, 3, 4)
    pblk = np.ascontiguousarray(pblk).reshape(NBLOCKS, PXB, 2)
    bxmin = pblk[:, :, 0].min(1)
    bxmax = pblk[:, :, 0].max(1)
    bymin = pblk[:, :, 1].min(1)
    bymax = pblk[:, :, 1].max(1)

    # distance from each sample point to each block bbox
    dx = np.maximum(np.maximum(bxmin[:, None] - q64[None, :, 0],
                               q64[None, :, 0] - bxmax[:, None]), 0.0)
    dy = np.maximum(np.maximum(bymin[:, None] - q64[None, :, 1],
                               q64[None, :, 1] - bymax[:, None]), 0.0)
    cand = dx * dx + dy * dy < (CUTOFF + 1e-3) ** 2  # [NBLOCKS, 512]
    kcnt = cand.sum(1)
    kpad = np.maximum(((kcnt + PADG - 1) // PADG) * PADG, PADG).astype(int)

    # ---- LPT assignment: exactly BLOCKS_PER_CORE blocks per core ----
    order = np.argsort(-kpad, kind="stable")
    loads = np.zeros(NCORES)
    counts = np.zeros(NCORES, dtype=int)
    assign = np.zeros(NBLOCKS, dtype=int)
    for b in order:
        elig = np.flatnonzero(counts < BLOCKS_PER_CORE)
        c = elig[np.argmin(loads[elig])]
        assign[b] = c
        loads[c] += kpad[b]
        counts[c] += 1

    # per-core slots sorted by descending kpad; shared schedule = slotwise max
    core_blocks = []
    for c in range(NCORES):
        blks = np.flatnonzero(assign == c)
        blks = blks[np.argsort(-kpad[blks], kind="stable")]
        core_blocks.append(blks)
    core_blocks = np.stack(core_blocks)  # [8, 32]
    k_sched = tuple(int(kpad[core_blocks[:, i]].max()) for i in range(BLOCKS_PER_CORE))

    if k_sched not in _prog_cache:
        _prog_cache.clear()
        _prog_cache[k_sched] = _build_program(k_sched)
    nc, mov_off = _prog_cache[k_sched]
    mov_total = int(mov_off[-1])

    # ---- moving-side limb rows (shared tables, gathered per block) ----
    q1x, q2x, q3x = _split3(q64[:, 0])
    q1y, q2y, q3y = _split3(q64[:, 1])
    qn1, qn2, qn3 = _split3(qn64)
    ones = np.ones(NPTS, dtype=np_bf16)
    mov_rows_all = np.stack(
        [q1x, q2x, q1x, q2x, q3x, q1x,
         q1y, q2y, q1y, q2y, q3y, q1y,
         ones, ones, ones,
         qn1, qn2, qn3]
    )  # [18, 512] bf16

    dum = np.float64(DUMMY[0])
    d1, d2, d3 = _split3(np.array([dum]))
    dn1, dn2, dn3 = _split3(np.array([2 * dum * dum]))
    mov_dummy = np.array(
        [d1[0], d2[0], d1[0], d2[0], d3[0], d1[0],
         d1[0], d2[0], d1[0], d2[0], d3[0], d1[0],
         1.0, 1.0, 1.0,
         dn1[0], dn2[0], dn3[0]], dtype=np_bf16)

    # ---- per-core input arrays ----
    in_maps = []
    for c in range(NCORES):
        pix = np.empty((KROWS, BLOCKS_PER_CORE * PXB), dtype=np_bf16)
        mov = np.empty((KROWS, mov_total), dtype=np_bf16)
        mov[:] = mov_dummy[:, None]
        for i, b in enumerate(core_blocks[c]):
            px = pblk[b].astype(np.float64)  # [1024, 2]
            sl = slice(i * PXB, (i + 1) * PXB)
            p1x, p2x, p3x = _split3(px[:, 0])
            p1y, p2y, p3y = _split3(px[:, 1])
            pn1, pn2, pn3 = _split3(px[:, 0] ** 2 + px[:, 1] ** 2)
            svx, _ = _limb_rows(p1x, p2x, p3x, None, None, None, scale=-2.0)
            svy, _ = _limb_rows(p1y, p2y, p3y, None, None, None, scale=-2.0)
            po = np.ones(PXB, dtype=np_bf16)
            pix[:, sl] = np.stack(svx + svy + [pn1, pn2, pn3, po, po, po])
            idx = np.flatnonzero(cand[b])
            o = int(mov_off[i])
            mov[:, o : o + len(idx)] = mov_rows_all[:, idx]
        in_maps.append({"pix": pix, "mov": mov})

    global _last_in_maps
    _last_in_maps = in_maps
    res = run_bass_kernel_spmd(nc, in_maps, core_ids=list(range(NCORES)))

    # ---- unshard: scatter block results back into the image ----
    img = np.empty(SIZE * SIZE, dtype=np.float32)
    by, bx = np.meshgrid(np.arange(NB), np.arange(NB), indexing="ij")
    lr, lc = np.meshgrid(np.arange(BLK), np.arange(BLK), indexing="ij")
    flat = ((by.reshape(-1, 1) * BLK + lr.reshape(-1)[None, :]) * SIZE
            + bx.reshape(-1, 1) * BLK + lc.reshape(-1)[None, :])  # [NBLOCKS, PXB]
    for c in range(NCORES):
        o = res.results[c]["out"].reshape(BLOCKS_PER_CORE, PXB)
        for i, b in enumerate(core_blocks[c]):
            img[flat[b]] = o[i]
    return img.reshape(1, SIZE, SIZE)
